# revision 1
# baseline (speedup 1.0000x reference)
"""Child-Sum Tree-LSTM (reference.py nn_ChildSumTreeLSTM) on 8 Trainium2
NeuronCores via Bass/Tile, SPMD.

Strategy: everything transposed (features on SBUF partitions, nodes on the
free dimension). Each core owns a contiguous slice of every level (levels
5..8); since children of a node are contiguous, the leaves->level-5
recursion is fully core-local (no collectives). The top levels (4..0,
341 nodes) are finished on the host in numpy during the gather step.

Matmuls run in bf16 (fp32 PSUM accumulation); the forget-gate fx term is
folded into the f-gate matmul via a step-0 broadcast rhs and all biases
ride in the activation instructions. The leaf level is computed in SBUF
groups consumed immediately by level-7 chunks (h/c never touch DRAM).
Emission is software-pipelined for the in-order TensorEngine, and the
child-sum runs incrementally on GpSimd as child chunks complete.
"""
import sys
sys.path.insert(0, '/opt/trn_rl_repo')
import numpy as np
import ml_dtypes
import concourse.bacc as bacc
import concourse.mybir as mybir
from concourse.tile import TileContext
from concourse.alu_op_type import AluOpType

F32 = mybir.dt.float32
BF16 = mybir.dt.bfloat16
AFT = mybir.ActivationFunctionType
P = 128
NCORES = 8
BR = 4


def level_offs(D):
    return [(BR ** l - 1) // (BR - 1) for l in range(D + 1)]


def local_counts(D, cut):
    return {l: BR ** l // NCORES for l in range(cut, D + 1)}


def local_offs(D, cut):
    n = local_counts(D, cut)
    offs = {}
    acc = 0
    for l in range(cut, D + 1):
        offs[l] = acc
        acc += n[l]
    return offs, acc


def build_program(D, cut, chunk=512, c_dtype=BF16, hs_gpsimd=True):
    nloc = local_counts(D, cut)
    loff, total_rows = local_offs(D, cut)
    CDT = c_dtype

    nc = bacc.Bacc("TRN2", target_bir_lowering=False, debug=False,
                   num_devices=NCORES)
    xT = nc.dram_tensor("xT", [2, P, total_rows], BF16, kind="ExternalInput")
    wx = nc.dram_tensor("wx", [2, P, 1024], BF16, kind="ExternalInput")
    wh = nc.dram_tensor("wh", [2, P, 1024], BF16, kind="ExternalInput")
    bias = nc.dram_tensor("bias", [P, 8], F32, kind="ExternalInput")
    ncut = nloc[cut]
    out_h = nc.dram_tensor("out_h", [2, P, ncut], BF16, kind="ExternalOutput")
    out_c = nc.dram_tensor("out_c", [2, P, ncut], CDT, kind="ExternalOutput")

    with TileContext(nc) as tc:
        with tc.tile_pool(name="const", bufs=1) as constp, \
             tc.tile_pool(name="xin", bufs=2) as xin, \
             tc.tile_pool(name="state", bufs=1) as statep, \
             tc.tile_pool(name="leafg", bufs=3) as leafg, \
             tc.tile_pool(name="work", bufs=2) as work, \
             tc.tile_pool(name="psum", bufs=4, space="PSUM") as psum:

            wxt = constp.tile([P, 2, 1024], BF16)
            wht = constp.tile([P, 2, 1024], BF16)
            bt = constp.tile([P, 8], F32)
            nc.sync.dma_start(wxt[:], wx[:].rearrange("a p n -> p a n"))
            nc.sync.dma_start(wht[:], wh[:].rearrange("a p n -> p a n"))
            nc.sync.dma_start(bt[:], bias[:])

            def load_x(l, c0, S, tag="xt", bufs=2):
                t = xin.tile([P, 2, S], BF16, tag=tag, bufs=bufs, name=tag)
                src = xT[:, :, loff[l] + c0: loff[l] + c0 + S]
                nc.sync.dma_start(t[:], src.rearrange("a p n -> p a n"))
                return t

            def gate_tiles(S, pfx=""):
                it = work.tile([P, 2, S], BF16, tag=pfx + "i", name="it")
                ot = work.tile([P, 2, S], BF16, tag=pfx + "o", name="ot")
                ut = work.tile([P, 2, S], BF16, tag=pfx + "u", name="ut")
                return it, ot, ut

            def iou_matmuls(xt, S, hs=None, ptag="ps", pbufs=3):
                """Returns list of 6 psum tiles [P, S] (i0,i1,o0,o1,u0,u1)."""
                out = []
                for mt in range(6):
                    ps = psum.tile([P, S], F32, tag=ptag, bufs=pbufs, name="ps")
                    nc.tensor.matmul(ps[:], wxt[:, 0, mt * P:(mt + 1) * P],
                                     xt[:, 0, :], start=True, stop=False)
                    last = hs is None
                    nc.tensor.matmul(ps[:], wxt[:, 1, mt * P:(mt + 1) * P],
                                     xt[:, 1, :], start=False, stop=last)
                    if hs is not None:
                        nc.tensor.matmul(ps[:], wht[:, 0, mt * P:(mt + 1) * P],
                                         hs[:, 0, :], start=False, stop=False)
                        nc.tensor.matmul(ps[:], wht[:, 1, mt * P:(mt + 1) * P],
                                         hs[:, 1, :], start=False, stop=True)
                    out.append(ps)
                return out

            def gates_from_psums(iou, it, ot, ut, S):
                for ft in range(2):
                    nc.scalar.activation(it[:, ft, :], iou[ft][:], AFT.Sigmoid,
                                         bias=bt[:, ft:ft + 1])
                    nc.scalar.activation(ot[:, ft, :], iou[2 + ft][:], AFT.Sigmoid,
                                         bias=bt[:, 2 + ft:3 + ft])
                    nc.scalar.activation(ut[:, ft, :], iou[4 + ft][:], AFT.Tanh,
                                         bias=bt[:, 4 + ft:5 + ft])

            def leaf_chunk(xt, S, h_dst, c_dst):
                iou = iou_matmuls(xt, S, ptag="psl")
                it, ot, ut = gate_tiles(S, pfx="l")
                gates_from_psums(iou, it, ot, ut, S)
                # fused over both ftiles
                with nc.allow_low_precision(reason="bf16 by design"):
                    nc.vector.tensor_tensor(c_dst, it[:], ut[:], AluOpType.mult)
                    nc.scalar.activation(ut[:], c_dst, AFT.Tanh)
                    nc.vector.tensor_tensor(h_dst, ot[:], ut[:], AluOpType.mult)

            def internal_chunk(l, c0, S, ch_h, ch_c, h_dst, c_dst, hs):
                xt = load_x(l, c0, S)
                # forget gates first: fh@child_h + fx@x_parent (broadcast rhs)
                nch = BR * S
                fw = min(1024, nch)          # f-psum width (<=2 banks)
                ft_tile = work.tile([P, 2, nch], BF16, tag="f", name="ft_tile")
                for ftt in range(2):
                    for q in range(nch // fw):
                        psf = psum.tile([P, fw], F32, tag="psf", bufs=1,
                                        name="psf")
                        for half in range(fw // 512) or [0]:
                            lo = q * fw + half * 512
                            w_ = min(512, nch - lo)
                            dst = psf[:, half * 512: half * 512 + w_]
                            nc.tensor.matmul(
                                dst, wht[:, 0, (768 + ftt * P):(768 + (ftt + 1) * P)],
                                ch_h[:, 0, lo:lo + w_], start=True, stop=False)
                            nc.tensor.matmul(
                                dst, wht[:, 1, (768 + ftt * P):(768 + (ftt + 1) * P)],
                                ch_h[:, 1, lo:lo + w_], start=False, stop=False)
                            plo, pw = lo // BR, w_ // BR
                            for kt in range(2):
                                rhs = xt[:, kt, plo:plo + pw] \
                                    .rearrange("p (n b) -> p n b", b=1) \
                                    .broadcast_to([P, pw, BR])
                                nc.tensor.matmul(
                                    dst.rearrange("p (n b) -> p n b", b=BR),
                                    wxt[:, kt, (768 + ftt * P):(768 + (ftt + 1) * P)],
                                    rhs, start=False, stop=(kt == 1))
                        nc.scalar.activation(ft_tile[:, ftt, q * fw:(q + 1) * fw],
                                             psf[:], AFT.Sigmoid,
                                             bias=bt[:, 6 + ftt:7 + ftt])
                # f * c_child (in place), group-sum into fcs
                fcs = work.tile([P, 2, S], CDT, tag="fcs", name="fcs")
                with nc.allow_low_precision(reason="bf16 by design"):
                    nc.vector.tensor_tensor(ft_tile[:], ft_tile[:], ch_c,
                                            AluOpType.mult)
                    for ft in range(2):
                        nc.vector.tensor_reduce(
                            fcs[:, ft, :],
                            ft_tile[:, ft, :].rearrange("p (n b) -> p n b", b=BR),
                            mybir.AxisListType.X, AluOpType.add)
                iou = iou_matmuls(xt, S, hs)
                it, ot, ut = gate_tiles(S)
                gates_from_psums(iou, it, ot, ut, S)
                with nc.allow_low_precision(reason="bf16 by design"):
                    # c = i*u + fcs ; h = o * tanh(c)   (ftile-fused)
                    nc.vector.tensor_tensor(it[:], it[:], ut[:], AluOpType.mult)
                    nc.vector.tensor_tensor(c_dst, it[:], fcs[:], AluOpType.add)
                    nc.scalar.activation(ut[:], c_dst, AFT.Tanh)
                    nc.vector.tensor_tensor(h_dst, ot[:], ut[:], AluOpType.mult)

            # ---- persistent level tiles ----
            lt_h = {}
            lt_c = {}
            for l in range(cut, D):
                lt_h[l] = statep.tile([P, 2, nloc[l]], BF16, tag=f"h{l}",
                                      name=f"h{l}")
                lt_c[l] = statep.tile([P, 2, nloc[l]], CDT, tag=f"c{l}",
                                      name=f"c{l}")
            # child-sum accumulators, filled incrementally as child h completes
            hs_t = {}
            for l in range(cut, D):
                hs_t[l] = statep.tile([P, 2, nloc[l]], BF16, tag=f"hs{l}",
                                      name=f"hs{l}")

            def emit_hsum(lpar, ch_ap, c0p, Sp):
                """Sum 4-child groups of ch_ap ([P,2,4*Sp]) into
                hs_t[lpar][:, :, c0p:c0p+Sp]."""
                with nc.allow_low_precision(reason="bf16 by design"):
                    htmp = work.tile([P, 2, Sp, 2], BF16, tag="htmp", name="htmp")
                    for ft in range(2):
                        v = ch_ap[:, ft, :].rearrange("p (n b) -> p n b", b=BR)
                        nc.gpsimd.tensor_add(htmp[:, ft, :, :],
                                             v[:, :, 0:2], v[:, :, 2:4])
                        nc.gpsimd.tensor_add(hs_t[lpar][:, ft, c0p:c0p + Sp],
                                             htmp[:, ft, :, 0],
                                             htmp[:, ft, :, 1])

            # ---- leaf level fused with level D-1 ----
            lp = D - 1
            pc = min(chunk, nloc[lp])
            n_groups = nloc[lp] // pc
            leafc = pc * BR
            pending = None
            for g in range(n_groups):
                h8g = leafg.tile([P, 2, leafc], BF16, tag="h8g", name="h8g")
                c8g = leafg.tile([P, 2, leafc], CDT, tag="c8g", name="c8g")
                lsub = min(chunk, leafc)
                for s in range(leafc // lsub):
                    xt = load_x(D, g * leafc + s * lsub, lsub, tag="xleaf",
                                bufs=4)
                    hsl = h8g[:, :, s * lsub:(s + 1) * lsub]
                    leaf_chunk(xt, lsub, hsl,
                               c8g[:, :, s * lsub:(s + 1) * lsub])
                    emit_hsum(lp, hsl, g * pc + s * lsub // BR, lsub // BR)
                if pending is not None:
                    internal_chunk(*pending)
                pending = (lp, g * pc, pc, h8g[:], c8g[:],
                           lt_h[lp][:, :, g * pc:(g + 1) * pc],
                           lt_c[lp][:, :, g * pc:(g + 1) * pc],
                           hs_t[lp][:, :, g * pc:(g + 1) * pc])
            internal_chunk(*pending)
            # ---- levels D-2 .. cut ----
            for l in range(D - 2, cut - 1, -1):
                # child-sum for this level's parents from level l+1 h
                emit_hsum(l, lt_h[l + 1][:], 0, nloc[l])
                S = nloc[l]
                pcS = min(chunk, S)
                for c0 in range(0, S, pcS):
                    internal_chunk(l, c0, pcS,
                                   lt_h[l + 1][:, :, c0 * BR:(c0 + pcS) * BR],
                                   lt_c[l + 1][:, :, c0 * BR:(c0 + pcS) * BR],
                                   lt_h[l][:, :, c0:c0 + pcS],
                                   lt_c[l][:, :, c0:c0 + pcS],
                                   hs_t[l][:, :, c0:c0 + pcS])

            nc.sync.dma_start(out_h[:].rearrange("a p n -> p a n"), lt_h[cut][:])
            nc.sync.dma_start(out_c[:].rearrange("a p n -> p a n"), lt_c[cut][:])

    nc.compile()
    return nc


def shard_inputs(x, W_iou_x, b_iou_x, W_iou_h, b_iou_h, W_fx, b_fx, W_fh, b_fh,
                 D, cut):
    offs = level_offs(D)
    nloc = local_counts(D, cut)
    wx_cat = np.concatenate([W_iou_x, W_fx], axis=0)
    wh_cat = np.concatenate([W_iou_h, W_fh], axis=0)
    wx_d = np.ascontiguousarray(wx_cat.T).reshape(2, P, 1024).astype(ml_dtypes.bfloat16)
    wh_d = np.ascontiguousarray(wh_cat.T).reshape(2, P, 1024).astype(ml_dtypes.bfloat16)
    b_iou = (b_iou_x + b_iou_h).reshape(6, P).T
    b_f = (b_fx + b_fh).reshape(2, P).T
    bias = np.ascontiguousarray(
        np.concatenate([b_iou, b_f], axis=1)).astype(np.float32)
    in_maps = []
    for k in range(NCORES):
        rows = []
        for l in range(cut, D + 1):
            n = nloc[l]
            rows.append(x[offs[l] + k * n: offs[l] + (k + 1) * n])
        xl = np.concatenate(rows, axis=0)
        xTk = np.ascontiguousarray(xl.T).reshape(2, P, -1).astype(ml_dtypes.bfloat16)
        in_maps.append({"xT": xTk, "wx": wx_d, "wh": wh_d, "bias": bias})
    return in_maps


def finish_host(results, x, W_iou_x, b_iou_x, W_iou_h, b_iou_h,
                W_fx, b_fx, W_fh, b_fh, D, cut):
    ncut = BR ** cut
    npc = ncut // NCORES
    Hc = np.empty((ncut, 256), np.float32)
    Cc = np.empty((ncut, 256), np.float32)
    for k in range(NCORES):
        oh = results[k]["out_h"].astype(np.float32).reshape(256, npc)
        oc = results[k]["out_c"].astype(np.float32).reshape(256, npc)
        Hc[k * npc:(k + 1) * npc] = oh.T
        Cc[k * npc:(k + 1) * npc] = oc.T
    sig = lambda v: 1.0 / (1.0 + np.exp(-v))
    h_next, c_next = Hc, Cc
    for l in range(cut - 1, -1, -1):
        n, off = BR ** l, (BR ** l - 1) // 3
        xl = x[off:off + n]
        child_h = h_next.reshape(n, BR, 256)
        child_c = c_next.reshape(n, BR, 256)
        chs = child_h.sum(axis=1)
        iou = xl @ W_iou_x.T + b_iou_x + chs @ W_iou_h.T + b_iou_h
        i, o, u = np.split(iou, 3, axis=1)
        i, o, u = sig(i), sig(o), np.tanh(u)
        f = sig(child_h @ W_fh.T + b_fh + (xl @ W_fx.T + b_fx)[:, None, :])
        c = i * u + (f * child_c).sum(axis=1)
        h = o * np.tanh(c)
        h_next, c_next = h, c
    return c_next.astype(np.float32), h_next.astype(np.float32)


# ---------------- public API ----------------

_D = 8
_CUT = 5
_CACHE = {}


def _get_program():
    if "nc" not in _CACHE:
        _CACHE["nc"] = build_program(_D, _CUT)
    return _CACHE["nc"]


def kernel(x, W_iou_x, b_iou_x, W_iou_h, b_iou_h, W_fx, b_fx, W_fh, b_fh):
    from concourse import bass_utils
    x = np.asarray(x, dtype=np.float32)
    args = [np.asarray(a, dtype=np.float32) for a in
            (W_iou_x, b_iou_x, W_iou_h, b_iou_h, W_fx, b_fx, W_fh, b_fh)]
    nc = _get_program()
    in_maps = shard_inputs(x, *args, _D, _CUT)
    res = bass_utils.run_bass_kernel_spmd(nc, in_maps,
                                          core_ids=list(range(NCORES)))
    c, h = finish_host(res.results, x, *args, _D, _CUT)
    return c, h



# revision 2
# speedup vs baseline: 1.0036x; 1.0036x over previous
"""Child-Sum Tree-LSTM (nn_ChildSumTreeLSTM) on 8 Trainium2 NeuronCores, v2.

Layout: transposed (features on partitions, nodes on free dim). Each core
owns 1/8 of levels 8 (leaves, 8192) and 7 (2048); levels 6..0 (5461 nodes,
3% of FLOPs) are finished on host in f32. All x is prefetched to SBUF;
leaf h/c persist in SBUF for the level-7 pass (nothing round-trips DRAM).

Per-chunk structure: leaf chunks of 1024 nodes (per-ft [P,1024] gate
activations), level-7 chunks of 512 parents. The forget-gate fx term is
computed once per parent (4 matmul cols/node) and broadcast-added into the
f psum on DVE before the sigmoid, replacing the baseline's 4x-replicated
broadcast matmuls. Child-h sums run on GpSimd; f*c uses a fused
mult + grouped tensor_reduce on DVE (2x bf16 mode). Level-7 emission lags
leaf emission by one group so the PE never waits on fresh activations.
"""
import sys
sys.path.insert(0, '/opt/trn_rl_repo')
import numpy as np
import ml_dtypes
import concourse.bacc as bacc
import concourse.mybir as mybir
from concourse.tile import TileContext
from concourse.alu_op_type import AluOpType

F32 = mybir.dt.float32
BF16 = mybir.dt.bfloat16
AFT = mybir.ActivationFunctionType
P = 128
NCORES = 8
BR = 4

NLEAF = 65536 // NCORES      # 8192 leaves per core
NL7 = 16384 // NCORES        # 2048 level-7 parents per core
SL = 1024                    # leaf chunk
S7 = 512                     # level-7 chunk
NLC = NLEAF // SL            # 8 leaf chunks
N7C = NL7 // S7              # 4 level-7 chunks




def build_program():
    nc = bacc.Bacc("TRN2", target_bir_lowering=False, debug=False,
                   num_devices=NCORES)
    total_rows = NLEAF + NL7
    xT = nc.dram_tensor("xT", [2, P, total_rows], BF16, kind="ExternalInput")
    wx = nc.dram_tensor("wx", [2, P, 1024], BF16, kind="ExternalInput")
    wh = nc.dram_tensor("wh", [2, P, 1024], BF16, kind="ExternalInput")
    bias = nc.dram_tensor("bias", [P, 8], F32, kind="ExternalInput")
    out_h = nc.dram_tensor("out_h", [2, P, NL7], BF16, kind="ExternalOutput")
    out_c = nc.dram_tensor("out_c", [2, P, NL7], BF16, kind="ExternalOutput")

    with TileContext(nc) as tc:
        with tc.tile_pool(name="const", bufs=1) as constp, \
             tc.tile_pool(name="state", bufs=1) as statep, \
             tc.tile_pool(name="work", bufs=2) as work, \
             tc.tile_pool(name="psum", bufs=1, space="PSUM") as psum:

            # ---- persistent tiles ----
            xleaf = statep.tile([P, 2, NLEAF], BF16)     # 32 KB/part
            xl7 = statep.tile([P, 2, NL7], BF16)         # 8 KB
            h8 = statep.tile([P, 2, NLEAF], BF16)        # 32 KB
            c8 = statep.tile([P, 2, NLEAF], BF16)        # 32 KB
            hs7 = statep.tile([P, 2, NL7], BF16)         # 8 KB
            wxt = constp.tile([P, 2, 1024], BF16)
            wht = constp.tile([P, 2, 1024], BF16)
            bt = constp.tile([P, 8], F32)

            # input DMAs: first leaf chunk's x first for a fast start
            LORD = [6, 7, 0, 1, 2, 3, 4, 5]      # leaf chunk order
            ORD7 = [3, 0, 1, 2]                  # level-7 chunk order
            c0 = LORD[0] * SL
            nc.sync.dma_start(xleaf[:, :, c0:c0 + SL],
                              xT[:, :, c0:c0 + SL].rearrange("a p n -> p a n"))
            nc.sync.dma_start(bt[:], bias[:])
            nc.sync.dma_start(wxt[:], wx[:].rearrange("a p n -> p a n"))
            c1 = LORD[1] * SL
            nc.sync.dma_start(xleaf[:, :, c1:c1 + SL],
                              xT[:, :, c1:c1 + SL].rearrange("a p n -> p a n"))
            for ls in LORD[2:]:
                a = ls * SL
                nc.sync.dma_start(
                    xleaf[:, :, a:a + SL],
                    xT[:, :, a:a + SL].rearrange("a p n -> p a n"))
            nc.sync.dma_start(wht[:], wh[:].rearrange("a p n -> p a n"))
            nc.sync.dma_start(
                xl7[:], xT[:, :, NLEAF:].rearrange("a p n -> p a n"))

            def gpsum(S):
                # one shared psum tag: [P, 2, 1024] f32 = 4 banks, 2 bufs
                t = psum.tile([P, 2, 1024], F32, tag="g", bufs=2, name="g")
                return t[:, :, :S] if S != 1024 else t

            def fpsum():
                t = psum.tile([P, 2, 1024], F32, tag="g", bufs=2, name="fps")
                return t.rearrange("p a n -> p (a n)")  # [P, 2048] view
                # (callers use [:, :1024])

            def gate_mm(ps, wt, col0, xt, S, extra=None):
                """ps[P,2,S] += wt[:,kt,col0:col0+128] @ xt[:,kt,:] (+extra)."""
                for ft in range(2):
                    ops = [(wt, col0 + ft * P, xt)]
                    if extra is not None:
                        ops.append((extra[0], extra[1] + ft * P, extra[2]))
                    n_mm = 2 * len(ops)
                    for h0 in range(0, S, 512):
                        w2 = min(512, S - h0)
                        k = 0
                        for w_, c_, x_ in ops:
                            for kt in range(2):
                                nc.tensor.matmul(
                                    ps[:, ft, h0:h0 + w2],
                                    w_[:, kt, c_:c_ + P],
                                    x_[:, kt, h0:h0 + w2],
                                    start=(k == 0),
                                    stop=(k == n_mm - 1))
                                k += 1

            def gate_act(dst, ps, func, bcol, S):
                for ft in range(2):
                    nc.scalar.activation(dst[:, ft, :S], ps[:, ft, :S], func,
                                         bias=bt[:, bcol + ft:bcol + ft + 1])

            def leaf_chunk(ls):
                b = ls * SL
                xt = xleaf[:, :, b:b + SL]
                it = work.tile([P, 2, SL], BF16, tag="it", name="it")
                ot = work.tile([P, 2, SL], BF16, tag="ot", name="ot")
                ut = work.tile([P, 2, SL], BF16, tag="ut", name="ut")
                for g, (dst, func, bc) in enumerate(
                        [(it, AFT.Sigmoid, 0), (ot, AFT.Sigmoid, 2),
                         (ut, AFT.Tanh, 4)]):
                    ps = gpsum(SL)
                    gate_mm(ps, wxt, g * 256, xt, SL)
                    gate_act(dst, ps, func, bc, SL)
                cs = c8[:, :, b:b + SL]
                hsl = h8[:, :, b:b + SL]
                with nc.allow_low_precision(reason="bf16 by design"):
                    nc.vector.tensor_tensor(cs, it[:], ut[:], AluOpType.mult)
                    nc.scalar.activation(ut[:], cs, AFT.Tanh)
                    nc.vector.tensor_tensor(hsl, ot[:], ut[:], AluOpType.mult)
                    # child-h sums for the SL//4 parents of this chunk (Pool)
                    pb, pn = b // BR, SL // BR
                    v = hsl.rearrange("p a (n b) -> p a n b", b=BR)
                    ht = work.tile([P, 2, pn, 2], BF16, tag="ht", name="ht")
                    nc.gpsimd.tensor_add(ht[:], v[:, :, :, 0:2], v[:, :, :, 2:4])
                    nc.gpsimd.tensor_add(hs7[:, :, pb:pb + pn],
                                         ht[:, :, :, 0], ht[:, :, :, 1])

            def l7_gates(c7i):
                """iou + fx matmuls/acts for level-7 chunk c7i (S7 parents)."""
                b = c7i * S7
                xt = xl7[:, :, b:b + S7]
                hst = hs7[:, :, b:b + S7]
                it = work.tile([P, 2, SL], BF16, tag="it", name="it")
                ot = work.tile([P, 2, SL], BF16, tag="ot", name="ot")
                ut = work.tile([P, 2, SL], BF16, tag="ut", name="ut")
                for g, (dst, func, bc) in enumerate(
                        [(it, AFT.Sigmoid, 0), (ot, AFT.Sigmoid, 2),
                         (ut, AFT.Tanh, 4)]):
                    ps = gpsum(S7)
                    gate_mm(ps, wxt, g * 256, xt, S7,
                            extra=(wht, g * 256, hst))
                    gate_act(dst, ps, func, bc, S7)
                # fx = W_fx @ x_parent, once per parent -> sbuf bf16
                fxs = work.tile([P, 2, S7], BF16, tag="fxs", name="fxs")
                ps = gpsum(S7)
                gate_mm(ps, wxt, 768, xt, S7)
                with nc.allow_low_precision(reason="bf16 by design"):
                    nc.vector.tensor_copy(fxs[:], ps[:, :, :S7])
                return it, ot, ut, fxs

            def l7_finish(c7i, it, ot, ut, fxs):
                """f gates, c/h for level-7 chunk c7i; DMA out."""
                b = c7i * S7
                cb = b * BR          # first child (leaf) index
                ftile = work.tile([P, 2, BR * S7], BF16, tag="f7", name="f7")
                for ftt in range(2):
                    for q in range(2):      # pieces of 1024 children
                        ps = fpsum()[:, :1024]
                        lo = cb + q * 1024
                        for h0 in (0, 512):
                            for kt in range(2):
                                nc.tensor.matmul(
                                    ps[:, h0:h0 + 512],
                                    wht[:, kt, 768 + ftt * P:768 + (ftt + 1) * P],
                                    h8[:, kt, lo + h0:lo + h0 + 512],
                                    start=(kt == 0), stop=(kt == 1))
                        # += fx broadcast over the 4 children (DVE)
                        with nc.allow_low_precision(reason="bf16 by design"):
                            nc.vector.tensor_tensor(
                                ps.rearrange("p (n b) -> p n b", b=BR),
                                ps.rearrange("p (n b) -> p n b", b=BR),
                                fxs[:, ftt, q * 256:(q + 1) * 256]
                                .rearrange("p (n b) -> p n b", b=1)
                                .broadcast_to([P, 256, BR]),
                                AluOpType.add)
                        nc.scalar.activation(
                            ftile[:, ftt, q * 1024:(q + 1) * 1024], ps,
                            AFT.Sigmoid, bias=bt[:, 6 + ftt:7 + ftt])
                fc2 = work.tile([P, 2, S7, 2], BF16, tag="fc2", name="fc2")
                fcs = work.tile([P, 2, S7], BF16, tag="fcs", name="fcs")
                c7t = work.tile([P, 2, S7], BF16, tag="c7t", name="c7t")
                h7t = work.tile([P, 2, S7], BF16, tag="h7t", name="h7t")
                with nc.allow_low_precision(reason="bf16 by design"):
                    nc.vector.tensor_tensor(ftile[:], ftile[:],
                                            c8[:, :, cb:cb + BR * S7],
                                            AluOpType.mult)
                    fv = ftile[:].rearrange("p a (n b) -> p a n b", b=BR)
                    nc.vector.tensor_tensor(fc2[:], fv[:, :, :, 0:2],
                                            fv[:, :, :, 2:4], AluOpType.add)
                    nc.vector.tensor_tensor(fcs[:], fc2[:, :, :, 0],
                                            fc2[:, :, :, 1], AluOpType.add)
                    nc.vector.tensor_tensor(it[:, :, :S7], it[:, :, :S7],
                                            ut[:, :, :S7], AluOpType.mult)
                    nc.vector.tensor_tensor(c7t[:], it[:, :, :S7], fcs[:],
                                            AluOpType.add)
                    nc.scalar.activation(ut[:, :, :S7], c7t[:], AFT.Tanh)
                    nc.vector.tensor_tensor(h7t[:], ot[:, :, :S7],
                                            ut[:, :, :S7], AluOpType.mult)
                nc.sync.dma_start(
                    out_h[:, :, b:b + S7].rearrange("a p n -> p a n"), h7t[:])
                nc.sync.dma_start(
                    out_c[:, :, b:b + S7].rearrange("a p n -> p a n"), c7t[:])

            # ---- schedule ----
            # Leaf chunks stream in LORD order; level-7 chunk ORD7[j]
            # (children = leaf chunks 2k, 2k+1, emitted at positions 2j,
            # 2j+1) starts one leaf chunk after its children finish, with
            # gates and f-phase interleaved between leaf chunks to keep
            # every engine fed.
            leaf_chunk(LORD[0])
            leaf_chunk(LORD[1])
            leaf_chunk(LORD[2])
            pend = None
            for j in range(N7C):
                k = ORD7[j]
                g = l7_gates(k)
                if 2 * j + 3 < NLC:
                    leaf_chunk(LORD[2 * j + 3])
                l7_finish(k, *g)
                if 2 * j + 4 < NLC:
                    leaf_chunk(LORD[2 * j + 4])

    nc.compile()
    return nc


def level_offs():
    return [(BR ** l - 1) // (BR - 1) for l in range(9)]


def shard_inputs(x, W_iou_x, b_iou_x, W_iou_h, b_iou_h, W_fx, b_fx, W_fh,
                 b_fh):
    offs = level_offs()
    wx_cat = np.concatenate([W_iou_x, W_fx], axis=0)
    wh_cat = np.concatenate([W_iou_h, W_fh], axis=0)
    wx_d = np.ascontiguousarray(wx_cat.T).reshape(2, P, 1024).astype(
        ml_dtypes.bfloat16)
    wh_d = np.ascontiguousarray(wh_cat.T).reshape(2, P, 1024).astype(
        ml_dtypes.bfloat16)
    b_iou = (b_iou_x + b_iou_h).reshape(6, P).T
    b_f = (b_fx + b_fh).reshape(2, P).T
    bias = np.ascontiguousarray(
        np.concatenate([b_iou, b_f], axis=1)).astype(np.float32)
    in_maps = []
    for k in range(NCORES):
        rows = [x[offs[8] + k * NLEAF: offs[8] + (k + 1) * NLEAF],
                x[offs[7] + k * NL7: offs[7] + (k + 1) * NL7]]
        xl = np.concatenate(rows, axis=0)
        xTk = np.ascontiguousarray(xl.T).reshape(2, P, -1).astype(
            ml_dtypes.bfloat16)
        in_maps.append({"xT": xTk, "wx": wx_d, "wh": wh_d, "bias": bias})
    return in_maps


def finish_host(results, x, W_iou_x, b_iou_x, W_iou_h, b_iou_h,
                W_fx, b_fx, W_fh, b_fh):
    n7 = 16384
    npc = n7 // NCORES
    Hc = np.empty((n7, 256), np.float32)
    Cc = np.empty((n7, 256), np.float32)
    for k in range(NCORES):
        oh = results[k]["out_h"].astype(np.float32).reshape(256, npc)
        oc = results[k]["out_c"].astype(np.float32).reshape(256, npc)
        Hc[k * npc:(k + 1) * npc] = oh.T
        Cc[k * npc:(k + 1) * npc] = oc.T
    sig = lambda v: 1.0 / (1.0 + np.exp(-v))
    h_next, c_next = Hc, Cc
    for l in range(6, -1, -1):
        n, off = BR ** l, (BR ** l - 1) // 3
        xl = x[off:off + n]
        child_h = h_next.reshape(n, BR, 256)
        child_c = c_next.reshape(n, BR, 256)
        chs = child_h.sum(axis=1)
        iou = xl @ W_iou_x.T + b_iou_x + chs @ W_iou_h.T + b_iou_h
        i, o, u = np.split(iou, 3, axis=1)
        i, o, u = sig(i), sig(o), np.tanh(u)
        fh = np.einsum('nbh,gh->nbg', child_h, W_fh)
        f = sig(fh + b_fh + (xl @ W_fx.T + b_fx)[:, None, :])
        c = i * u + (f * child_c).sum(axis=1)
        h = o * np.tanh(c)
        h_next, c_next = h, c
    return c_next.astype(np.float32), h_next.astype(np.float32)


# ---------------- public API ----------------

_CACHE = {}


def _get_program():
    if "nc" not in _CACHE:
        _CACHE["nc"] = build_program()
    return _CACHE["nc"]


def kernel(x, W_iou_x, b_iou_x, W_iou_h, b_iou_h, W_fx, b_fx, W_fh, b_fh):
    from concourse import bass_utils
    x = np.asarray(x, dtype=np.float32)
    args = [np.asarray(a, dtype=np.float32) for a in
            (W_iou_x, b_iou_x, W_iou_h, b_iou_h, W_fx, b_fx, W_fh, b_fh)]
    nc = _get_program()
    in_maps = shard_inputs(x, *args)
    res = bass_utils.run_bass_kernel_spmd(nc, in_maps,
                                          core_ids=list(range(NCORES)))
    c, h = finish_host(res.results, x, *args)
    return c, h


# revision 3
# speedup vs baseline: 1.0153x; 1.0118x over previous
"""Child-Sum Tree-LSTM (nn_ChildSumTreeLSTM) on 8 Trainium2 NeuronCores, v2.

Layout: transposed (features on partitions, nodes on free dim). Each core
owns 1/8 of levels 8 (leaves, 8192) and 7 (2048); levels 6..0 (5461 nodes,
3% of FLOPs) are finished on host in f32. All x is prefetched to SBUF;
leaf h/c persist in SBUF for the level-7 pass (nothing round-trips DRAM).

Per-chunk structure: leaf chunks of 1024 nodes (per-ft [P,1024] gate
activations), level-7 chunks of 512 parents. The forget-gate fx term is
computed once per parent (4 matmul cols/node) and broadcast-added into the
f psum on DVE before the sigmoid, replacing the baseline's 4x-replicated
broadcast matmuls. Child-h sums run on GpSimd; f*c uses a fused
mult + grouped tensor_reduce on DVE (2x bf16 mode). Level-7 emission lags
leaf emission by one group so the PE never waits on fresh activations.
"""
import sys
sys.path.insert(0, '/opt/trn_rl_repo')
import numpy as np
import ml_dtypes
import concourse.bacc as bacc
import concourse.mybir as mybir
from concourse.tile import TileContext
from concourse.alu_op_type import AluOpType

F32 = mybir.dt.float32
BF16 = mybir.dt.bfloat16
AFT = mybir.ActivationFunctionType
P = 128
NCORES = 8
BR = 4

NLEAF = 65536 // NCORES      # 8192 leaves per core
NL7 = 16384 // NCORES        # 2048 level-7 parents per core
SL = 1024                    # leaf chunk
S7 = 512                     # level-7 chunk
NLC = NLEAF // SL            # 8 leaf chunks
N7C = NL7 // S7              # 4 level-7 chunks




def build_program():
    nc = bacc.Bacc("TRN2", target_bir_lowering=False, debug=False,
                   num_devices=NCORES)
    total_rows = NLEAF + NL7
    xT = nc.dram_tensor("xT", [2, P, total_rows], BF16, kind="ExternalInput")
    wx = nc.dram_tensor("wx", [2, P, 1024], BF16, kind="ExternalInput")
    wh = nc.dram_tensor("wh", [2, P, 1024], BF16, kind="ExternalInput")
    bias = nc.dram_tensor("bias", [P, 8], F32, kind="ExternalInput")
    out_h = nc.dram_tensor("out_h", [2, P, NL7], BF16, kind="ExternalOutput")
    out_c = nc.dram_tensor("out_c", [2, P, NL7], BF16, kind="ExternalOutput")

    with TileContext(nc) as tc:
        with tc.tile_pool(name="const", bufs=1) as constp, \
             tc.tile_pool(name="state", bufs=1) as statep, \
             tc.tile_pool(name="work", bufs=2) as work, \
             tc.tile_pool(name="psum", bufs=1, space="PSUM") as psum:

            # ---- persistent tiles ----
            xleaf = statep.tile([P, 2, NLEAF], BF16)     # 32 KB/part
            xl7 = statep.tile([P, 2, NL7], BF16)         # 8 KB
            h8 = statep.tile([P, 2, NLEAF], BF16)        # 32 KB
            c8 = statep.tile([P, 2, NLEAF], BF16)        # 32 KB
            hs7 = statep.tile([P, 2, NL7], BF16)         # 8 KB
            wxt = constp.tile([P, 2, 1024], BF16)
            wht = constp.tile([P, 2, 1024], BF16)
            bt = constp.tile([P, 8], F32)

            # input DMAs: first leaf chunk's x first for a fast start
            LORD = [6, 7, 0, 1, 2, 3, 4, 5]      # leaf chunk order
            ORD7 = [3, 0, 1, 2]                  # level-7 chunk order
            c0 = LORD[0] * SL
            nc.sync.dma_start(xleaf[:, :, c0:c0 + SL],
                              xT[:, :, c0:c0 + SL].rearrange("a p n -> p a n"))
            nc.sync.dma_start(bt[:], bias[:])
            nc.sync.dma_start(wxt[:, :, 0:768],
                              wx[:, :, 0:768].rearrange("a p n -> p a n"))
            c1 = LORD[1] * SL
            nc.sync.dma_start(xleaf[:, :, c1:c1 + SL],
                              xT[:, :, c1:c1 + SL].rearrange("a p n -> p a n"))
            for ls in LORD[2:]:
                a = ls * SL
                nc.sync.dma_start(
                    xleaf[:, :, a:a + SL],
                    xT[:, :, a:a + SL].rearrange("a p n -> p a n"))
            nc.sync.dma_start(wxt[:, :, 768:1024],
                              wx[:, :, 768:1024].rearrange("a p n -> p a n"))
            nc.sync.dma_start(wht[:], wh[:].rearrange("a p n -> p a n"))
            nc.sync.dma_start(
                xl7[:], xT[:, :, NLEAF:].rearrange("a p n -> p a n"))

            def gpsum(S):
                # one shared psum tag: [P, 2, 1024] f32 = 4 banks, 2 bufs
                t = psum.tile([P, 2, 1024], F32, tag="g", bufs=2, name="g")
                return t[:, :, :S] if S != 1024 else t

            def fpsum():
                t = psum.tile([P, 2, 1024], F32, tag="g", bufs=2, name="fps")
                return t.rearrange("p a n -> p (a n)")  # [P, 2048] view
                # (callers use [:, :1024])

            def gate_mm(ps, wt, col0, xt, S, extra=None):
                """ps[P,2,S] += wt[:,kt,col0:col0+128] @ xt[:,kt,:] (+extra)."""
                for ft in range(2):
                    ops = [(wt, col0 + ft * P, xt)]
                    if extra is not None:
                        ops.append((extra[0], extra[1] + ft * P, extra[2]))
                    n_mm = 2 * len(ops)
                    k = 0
                    for w_, c_, x_ in ops:
                        for kt in range(2):
                            # h0 innermost: consecutive matmuls share the
                            # loaded weight tile (halves the LDWEIGHTS count)
                            for h0 in range(0, S, 512):
                                w2 = min(512, S - h0)
                                nc.tensor.matmul(
                                    ps[:, ft, h0:h0 + w2],
                                    w_[:, kt, c_:c_ + P],
                                    x_[:, kt, h0:h0 + w2],
                                    start=(k == 0),
                                    stop=(k == n_mm - 1))
                            k += 1

            def gate_act(dst, ps, func, bcol, S):
                for ft in range(2):
                    nc.scalar.activation(dst[:, ft, :S], ps[:, ft, :S], func,
                                         bias=bt[:, bcol + ft:bcol + ft + 1])

            def leaf_chunk(ls):
                b = ls * SL
                xt = xleaf[:, :, b:b + SL]
                it = work.tile([P, 2, SL], BF16, tag="it", name="it")
                ot = work.tile([P, 2, SL], BF16, tag="ot", name="ot")
                ut = work.tile([P, 2, SL], BF16, tag="ut", name="ut")
                for g, (dst, func, bc) in enumerate(
                        [(it, AFT.Sigmoid, 0), (ot, AFT.Sigmoid, 2),
                         (ut, AFT.Tanh, 4)]):
                    ps = gpsum(SL)
                    gate_mm(ps, wxt, g * 256, xt, SL)
                    gate_act(dst, ps, func, bc, SL)
                cs = c8[:, :, b:b + SL]
                hsl = h8[:, :, b:b + SL]
                with nc.allow_low_precision(reason="bf16 by design"):
                    nc.vector.tensor_tensor(cs, it[:], ut[:], AluOpType.mult)
                    nc.scalar.activation(ut[:], cs, AFT.Tanh)
                    nc.vector.tensor_tensor(hsl, ot[:], ut[:], AluOpType.mult)
                    # child-h sums for the SL//4 parents of this chunk (Pool)
                    pb, pn = b // BR, SL // BR
                    v = hsl.rearrange("p a (n b) -> p a n b", b=BR)
                    ht = work.tile([P, 2, pn, 2], BF16, tag="ht", name="ht")
                    nc.gpsimd.tensor_add(ht[:], v[:, :, :, 0:2], v[:, :, :, 2:4])
                    nc.gpsimd.tensor_add(hs7[:, :, pb:pb + pn],
                                         ht[:, :, :, 0], ht[:, :, :, 1])

            def l7_gates(c7i):
                """iou + fx matmuls/acts for level-7 chunk c7i (S7 parents)."""
                b = c7i * S7
                xt = xl7[:, :, b:b + S7]
                hst = hs7[:, :, b:b + S7]
                it = work.tile([P, 2, SL], BF16, tag="it", name="it")
                ot = work.tile([P, 2, SL], BF16, tag="ot", name="ot")
                ut = work.tile([P, 2, SL], BF16, tag="ut", name="ut")
                for g, (dst, func, bc) in enumerate(
                        [(it, AFT.Sigmoid, 0), (ot, AFT.Sigmoid, 2),
                         (ut, AFT.Tanh, 4)]):
                    ps = gpsum(S7)
                    gate_mm(ps, wxt, g * 256, xt, S7,
                            extra=(wht, g * 256, hst))
                    gate_act(dst, ps, func, bc, S7)
                # fx = W_fx @ x_parent, once per parent -> sbuf bf16
                fxs = work.tile([P, 2, S7], BF16, tag="fxs", name="fxs")
                ps = gpsum(S7)
                gate_mm(ps, wxt, 768, xt, S7)
                with nc.allow_low_precision(reason="bf16 by design"):
                    nc.vector.tensor_copy(fxs[:], ps[:, :, :S7])
                return it, ot, ut, fxs

            def l7_finish(c7i, it, ot, ut, fxs):
                """f gates, c/h for level-7 chunk c7i; DMA out."""
                b = c7i * S7
                cb = b * BR          # first child (leaf) index
                ftile = work.tile([P, 2, BR * S7], BF16, tag="f7", name="f7")
                for ftt in range(2):
                    for q in range(2):      # pieces of 1024 children
                        ps = fpsum()[:, :1024]
                        lo = cb + q * 1024
                        for kt in range(2):
                            for h0 in (0, 512):
                                nc.tensor.matmul(
                                    ps[:, h0:h0 + 512],
                                    wht[:, kt, 768 + ftt * P:768 + (ftt + 1) * P],
                                    h8[:, kt, lo + h0:lo + h0 + 512],
                                    start=(kt == 0), stop=(kt == 1))
                        # += fx broadcast over the 4 children (DVE)
                        with nc.allow_low_precision(reason="bf16 by design"):
                            nc.vector.tensor_tensor(
                                ps.rearrange("p (n b) -> p n b", b=BR),
                                ps.rearrange("p (n b) -> p n b", b=BR),
                                fxs[:, ftt, q * 256:(q + 1) * 256]
                                .rearrange("p (n b) -> p n b", b=1)
                                .broadcast_to([P, 256, BR]),
                                AluOpType.add)
                        nc.scalar.activation(
                            ftile[:, ftt, q * 1024:(q + 1) * 1024], ps,
                            AFT.Sigmoid, bias=bt[:, 6 + ftt:7 + ftt])
                fc2 = work.tile([P, 2, S7, 2], BF16, tag="fc2", name="fc2")
                fcs = work.tile([P, 2, S7], BF16, tag="fcs", name="fcs")
                c7t = work.tile([P, 2, S7], BF16, tag="c7t", name="c7t")
                h7t = work.tile([P, 2, S7], BF16, tag="h7t", name="h7t")
                with nc.allow_low_precision(reason="bf16 by design"):
                    nc.vector.tensor_tensor(ftile[:], ftile[:],
                                            c8[:, :, cb:cb + BR * S7],
                                            AluOpType.mult)
                    fv = ftile[:].rearrange("p a (n b) -> p a n b", b=BR)
                    nc.vector.tensor_tensor(fc2[:], fv[:, :, :, 0:2],
                                            fv[:, :, :, 2:4], AluOpType.add)
                    nc.vector.tensor_tensor(fcs[:], fc2[:, :, :, 0],
                                            fc2[:, :, :, 1], AluOpType.add)
                    nc.vector.tensor_tensor(it[:, :, :S7], it[:, :, :S7],
                                            ut[:, :, :S7], AluOpType.mult)
                    nc.vector.tensor_tensor(c7t[:], it[:, :, :S7], fcs[:],
                                            AluOpType.add)
                    nc.scalar.activation(ut[:, :, :S7], c7t[:], AFT.Tanh)
                    nc.vector.tensor_tensor(h7t[:], ot[:, :, :S7],
                                            ut[:, :, :S7], AluOpType.mult)
                nc.sync.dma_start(
                    out_h[:, :, b:b + S7].rearrange("a p n -> p a n"), h7t[:])
                nc.sync.dma_start(
                    out_c[:, :, b:b + S7].rearrange("a p n -> p a n"), c7t[:])

            # ---- schedule ----
            # Leaf chunks stream in LORD order; level-7 chunk ORD7[j]
            # (children = leaf chunks 2k, 2k+1, emitted at positions 2j,
            # 2j+1) starts one leaf chunk after its children finish, with
            # gates and f-phase interleaved between leaf chunks to keep
            # every engine fed.
            leaf_chunk(LORD[0])
            leaf_chunk(LORD[1])
            leaf_chunk(LORD[2])
            pend = None
            for j in range(N7C):
                k = ORD7[j]
                g = l7_gates(k)
                if 2 * j + 3 < NLC:
                    leaf_chunk(LORD[2 * j + 3])
                l7_finish(k, *g)
                if 2 * j + 4 < NLC:
                    leaf_chunk(LORD[2 * j + 4])

    nc.compile()
    return nc


def level_offs():
    return [(BR ** l - 1) // (BR - 1) for l in range(9)]


def shard_inputs(x, W_iou_x, b_iou_x, W_iou_h, b_iou_h, W_fx, b_fx, W_fh,
                 b_fh):
    offs = level_offs()
    wx_cat = np.concatenate([W_iou_x, W_fx], axis=0)
    wh_cat = np.concatenate([W_iou_h, W_fh], axis=0)
    wx_d = np.ascontiguousarray(wx_cat.T).reshape(2, P, 1024).astype(
        ml_dtypes.bfloat16)
    wh_d = np.ascontiguousarray(wh_cat.T).reshape(2, P, 1024).astype(
        ml_dtypes.bfloat16)
    b_iou = (b_iou_x + b_iou_h).reshape(6, P).T
    b_f = (b_fx + b_fh).reshape(2, P).T
    bias = np.ascontiguousarray(
        np.concatenate([b_iou, b_f], axis=1)).astype(np.float32)
    in_maps = []
    for k in range(NCORES):
        rows = [x[offs[8] + k * NLEAF: offs[8] + (k + 1) * NLEAF],
                x[offs[7] + k * NL7: offs[7] + (k + 1) * NL7]]
        xl = np.concatenate(rows, axis=0)
        xTk = np.ascontiguousarray(xl.T).reshape(2, P, -1).astype(
            ml_dtypes.bfloat16)
        in_maps.append({"xT": xTk, "wx": wx_d, "wh": wh_d, "bias": bias})
    return in_maps


def finish_host(results, x, W_iou_x, b_iou_x, W_iou_h, b_iou_h,
                W_fx, b_fx, W_fh, b_fh):
    n7 = 16384
    npc = n7 // NCORES
    Hc = np.empty((n7, 256), np.float32)
    Cc = np.empty((n7, 256), np.float32)
    for k in range(NCORES):
        oh = results[k]["out_h"].astype(np.float32).reshape(256, npc)
        oc = results[k]["out_c"].astype(np.float32).reshape(256, npc)
        Hc[k * npc:(k + 1) * npc] = oh.T
        Cc[k * npc:(k + 1) * npc] = oc.T
    sig = lambda v: 1.0 / (1.0 + np.exp(-v))
    h_next, c_next = Hc, Cc
    for l in range(6, -1, -1):
        n, off = BR ** l, (BR ** l - 1) // 3
        xl = x[off:off + n]
        child_h = h_next.reshape(n, BR, 256)
        child_c = c_next.reshape(n, BR, 256)
        chs = child_h.sum(axis=1)
        iou = xl @ W_iou_x.T + b_iou_x + chs @ W_iou_h.T + b_iou_h
        i, o, u = np.split(iou, 3, axis=1)
        i, o, u = sig(i), sig(o), np.tanh(u)
        fh = np.einsum('nbh,gh->nbg', child_h, W_fh)
        f = sig(fh + b_fh + (xl @ W_fx.T + b_fx)[:, None, :])
        c = i * u + (f * child_c).sum(axis=1)
        h = o * np.tanh(c)
        h_next, c_next = h, c
    return c_next.astype(np.float32), h_next.astype(np.float32)


# ---------------- public API ----------------

_CACHE = {}


def _get_program():
    if "nc" not in _CACHE:
        _CACHE["nc"] = build_program()
    return _CACHE["nc"]


def kernel(x, W_iou_x, b_iou_x, W_iou_h, b_iou_h, W_fx, b_fx, W_fh, b_fh):
    from concourse import bass_utils
    x = np.asarray(x, dtype=np.float32)
    args = [np.asarray(a, dtype=np.float32) for a in
            (W_iou_x, b_iou_x, W_iou_h, b_iou_h, W_fx, b_fx, W_fh, b_fh)]
    nc = _get_program()
    in_maps = shard_inputs(x, *args)
    res = bass_utils.run_bass_kernel_spmd(nc, in_maps,
                                          core_ids=list(range(NCORES)))
    c, h = finish_host(res.results, x, *args)
    return c, h
